# revision 35
# baseline (speedup 1.0000x reference)
"""Trainium2 Bass kernel for nn_Attention_60885456388891 (gnn_message_passing).

Computation (per batch b):
  node_h = h @ W_h2node + b_h2node
  score_n[n] = sum_d tanh(p_node_feats[b,n,d] + node_h[b,d]) * w_alpha1[d]
  node_w = renorm(softmax(score_n) * att_masks)
  node_res_ = sum_n node_w[n] * node_feats[b,n,:]
  (same for relations)
  node_res = glu(cat(node_res_, rela_res_) @ W_ng + b_ng)
  rela_res = glu(cat(rela_res_, node_res) @ W_rg + b_rg)

Strategy: pure data-parallel over batch B=512 across 8 cores (64 batches/core),
all features downcast to fp16 on the host (halves HBM traffic; rel-err ~1e-3
vs the 2e-2 gate).

Per-core pipeline (v4 design):
  - pnf/prf streamed in d-on-partitions layout: broadcast-add of node_h/rela_h
    becomes a per-partition-scalar DVE add (fp16 4x-ish mode), tanh batches
    into large ACT calls.
  - scores via tiny tanh-stationary PE matmuls (FWL fast weight load):
    lhsT = tanh chunk [128d, 128n], rhs = w_alpha chunk col -> score columns
    accumulate in PSUM.  |score| <= sum|w_alpha| ~ 8, so exp() needs no max
    subtraction; masked-exp columns are used UNNORMALIZED as weights, and the
    1/sum(EM) normalizer is folded into the GLU epilogue as a per-row scale.
  - phase C: nf/rf chunk stationary [128n, 128d] x EM column -> X^T columns
    accumulate directly in the k-chunked layout the GLU matmuls consume (no
    row staging, no transposes).
  - GLU: out = (Xn_u @ Wtop) * rSn + (Xr_u @ Wbot) * rSr + bias_bcast,
    a*sigmoid(g).
"""

import numpy as np

import concourse.bass as bass
import concourse.bacc as bacc
import concourse.mybir as mybir
import concourse.tile as tile
from concourse.bass_utils import run_bass_kernel_spmd

# Problem dims (hardcoded per contract)
B, N, R, D = 512, 128, 256, 512
NCORES = 8
BS = B // NCORES          # 64 batches per core
PAIR = 4                  # batches per stream DMA block
NBLK = BS // PAIR         # 16 blocks
G = 16                    # batches per softmax/psum group
GROUPS = BS // G          # 4 groups
KC = D // 128             # 4 k-chunks of 128
KC2 = 2 * D // 128        # 8 k-chunks for the 1024-wide GLU matmuls
VP = PAIR                 # batches per value-stream DMA block
NVBLK = BS // VP          # value blocks
VPJ = G // VP             # value blocks per group

F32 = mybir.dt.float32
F16 = mybir.dt.float16
AF = mybir.ActivationFunctionType
ALU = mybir.AluOpType
AX = mybir.AxisListType


def build_program():
    nc = bacc.Bacc("TRN2", target_bir_lowering=False, debug=False)

    def din(name, shape, dt=F16):
        return nc.dram_tensor(name, shape, dt, kind="ExternalInput").ap()

    h_d = din("h", [BS, D])
    pnf_d = din("pnf", [NBLK, 128, PAIR, KC, N])        # d-partition args
    prf_d = din("prf", [NBLK, 128, PAIR, KC, R])
    nf_d = din("nf", [NVBLK, 128, VP, KC, 128])         # n-partition values
    rf_d = din("rf", [NVBLK, 128, VP, 2, KC, 128])
    mT_d = din("mT", [128, 3, BS])                      # masks, transposed
    Wn_d = din("w_h2node", [128, KC, D])
    bn_d = din("b_h2node", [128, KC], F32)
    Wr_d = din("w_h2rela", [128, KC, D])
    br_d = din("b_h2rela", [128, KC], F32)
    w1_d = din("w1c", [128, KC])                        # w_alpha1 as columns
    w2_d = din("w2c", [128, KC])
    Wng_d = din("w_ng", [128, KC2, 2, 512])
    Wrg_d = din("w_rg", [128, KC2, 2, 512])
    bng_d = din("bias_ng", [BS, 2, 512], F32)           # host-broadcast bias
    brg_d = din("bias_rg", [BS, 2, 512], F32)
    id_d = din("ident", [128, 128])                     # f16 identity
    ones_d = din("ones_col", [128, 1])                  # f16 ones column
    onesr_d = din("ones_row", [1, 128])                 # f16 ones row

    nres_d = nc.dram_tensor("node_res", [BS, D], F32, kind="ExternalOutput").ap()
    rres_d = nc.dram_tensor("rela_res", [BS, D], F32, kind="ExternalOutput").ap()

    dma = nc.sync.dma_start
    dma_s = nc.gpsimd.dma_start

    with tile.TileContext(nc) as tc:
        with (
            tc.tile_pool(name="const", bufs=1) as cp,
        ):
            # ---- persistent constants ----
            ident = cp.tile([128, 128], F16)
            dma(out=ident, in_=id_d)
            ones_col = cp.tile([128, 1], F16)
            dma(out=ones_col, in_=ones_d)
            ones_row = cp.tile([1, 128], F16)
            dma(out=ones_row, in_=onesr_d)
            w1c = cp.tile([128, KC], F16)
            dma(out=w1c, in_=w1_d)
            w2c = cp.tile([128, KC], F16)
            dma(out=w2c, in_=w2_d)
            mT = cp.tile([128, 3, BS], F16)
            dma(out=mT, in_=mT_d)

            # persistent outputs of phase C / B
            XTn = cp.tile([128, KC, BS], F16, tag="xtn")   # unnormalized Xn^T
            XTr = cp.tile([128, KC, BS], F16, tag="xtr")
            S_sb = cp.tile([1, 3, BS], F32, tag="ssb")     # EM column sums
            nhT = cp.tile([128, KC, BS], F32, tag="nht")   # bias columns
            rhT = cp.tile([128, KC, BS], F32, tag="rht")
            # GLU weights/biases (DMA'd during the last group's streaming)
            Wng_sb = cp.tile([128, KC2, 2, 512], F16, tag="wng")
            Wrg_sb = cp.tile([128, KC2, 2, 512], F16, tag="wrg")
            bng_sb = cp.tile([BS, 2, 512], F32, tag="bng")
            brg_sb = cp.tile([BS, 2, 512], F32, tag="brg")

            # ---- prologue: nhT/rhT bias columns = (h @ W + b)^T, computed
            # directly in transposed chunk layout (no row round-trip) ----
            with (
                tc.tile_pool(name="prol", bufs=1) as pp,
                tc.tile_pool(name="prps", bufs=2, space="PSUM") as pps,
            ):
                h_sb = pp.tile([BS, D], F16, tag="h")
                dma(out=h_sb, in_=h_d)
                Wn_sb = pp.tile([128, KC, D], F16, tag="wn")
                dma(out=Wn_sb, in_=Wn_d)
                Wr_sb = pp.tile([128, KC, D], F16, tag="wr")
                dma(out=Wr_sb, in_=Wr_d)
                bn_sb = pp.tile([128, KC], F32, tag="bn")
                dma(out=bn_sb, in_=bn_d)
                br_sb = pp.tile([128, KC], F32, tag="br")
                dma(out=br_sb, in_=br_d)

                hT = pp.tile([128, KC, BS], F16, tag="ht")
                for c in range(KC):
                    tps = pps.tile([128, BS], F16, tag="tps")
                    nc.tensor.transpose(tps, h_sb[:, c * 128:(c + 1) * 128],
                                        ident[:BS, :BS])
                    nc.vector.tensor_copy(hT[:, c, :], tps)
                for W_sb, b_sb, dstT in ((Wn_sb, bn_sb, nhT),
                                         (Wr_sb, br_sb, rhT)):
                    for c in range(KC):
                        ps = pps.tile([128, BS], F32, tag="nhcps")
                        for k in range(KC):
                            nc.tensor.matmul(
                                ps, W_sb[:, k, c * 128:(c + 1) * 128],
                                hT[:, k, :],
                                start=(k == 0), stop=(k == KC - 1))
                        nc.vector.tensor_scalar_add(dstT[:, c, :], ps,
                                                    b_sb[:, c:c + 1])

            # ---- main loop: per-PAIR software pipeline over 16 slots ----
            # Slot k: [pnf/prf DMA (k)] [exp/mask/S (k-1)] [phase-C mm (k-2)]
            #         [adds+tanh (k)] [score mm (k)] [nf/rf prefetch DMA (k)]
            with (
                tc.tile_pool(name="pnfp", bufs=5) as pnfp,
                tc.tile_pool(name="prfp", bufs=5) as prfp,
                tc.tile_pool(name="nfp", bufs=6) as nfp,
                tc.tile_pool(name="rfp", bufs=6) as rfp,
                tc.tile_pool(name="emp", bufs=2) as emp,
                tc.tile_pool(name="scps", bufs=2, space="PSUM") as scps,
                tc.tile_pool(name="xps", bufs=2, space="PSUM") as xps,
                tc.tile_pool(name="sps", bufs=2, space="PSUM") as sps,
            ):
                NPJ = G // PAIR       # pair slots per group
                NSLOT = NBLK          # total slots
                LAG = 2               # phase-C runs LAG slots behind phase-A
                st = {}               # per-slot live tiles

                def phase_a(k):
                    pnf = pnfp.tile([128, PAIR, KC, N], F16, tag="pnf")
                    dma(out=pnf, in_=pnf_d[k])
                    prf = prfp.tile([128, PAIR, KC, R], F16, tag="prf")
                    dma(out=prf, in_=prf_d[k])
                    st[("pnf", k)] = pnf
                    st[("prf", k)] = prf

                def phase_a_compute(k, sc, j):
                    pnf = st.pop(("pnf", k))
                    prf = st.pop(("prf", k))
                    for i in range(PAIR):
                        b = k * PAIR + i
                        for c in range(KC):
                            nc.vector.tensor_scalar_add(
                                pnf[:, i, c, :], pnf[:, i, c, :],
                                nhT[:, c, b:b + 1])
                            nc.vector.tensor_scalar_add(
                                prf[:, i, c, :], prf[:, i, c, :],
                                rhT[:, c, b:b + 1])
                    nc.scalar.activation(pnf, pnf, AF.Tanh)
                    nc.scalar.activation(prf, prf, AF.Tanh)
                    for i in range(PAIR):
                        jj = j * PAIR + i
                        for c in range(KC):
                            nc.tensor.matmul(
                                sc[:, 0, jj:jj + 1], pnf[:, i, c, :],
                                w1c[:, c:c + 1],
                                start=(c == 0), stop=(c == KC - 1))
                        for c in range(KC):
                            nc.tensor.matmul(
                                sc[:, 1, jj:jj + 1], prf[:, i, c, :128],
                                w2c[:, c:c + 1],
                                start=(c == 0), stop=(c == KC - 1))
                        for c in range(KC):
                            nc.tensor.matmul(
                                sc[:, 2, jj:jj + 1], prf[:, i, c, 128:],
                                w2c[:, c:c + 1],
                                start=(c == 0), stop=(c == KC - 1))

                def phase_b(g, sc):
                    """masked exp + column sums for group g."""
                    g0 = g * G
                    em = emp.tile([128, 3, G], F16, tag="em")
                    st[("em", g)] = em
                    nc.scalar.activation(em, sc, AF.Exp)
                    nc.vector.tensor_mul(em, em, mT[:, :, g0:g0 + G])
                    s_ps = sps.tile([1, 3, G], F32, tag="s")
                    nc.tensor.matmul(s_ps, ones_col, em, start=True, stop=True)
                    nc.vector.tensor_copy(S_sb[:, :, g0:g0 + G], s_ps)

                def prefetch_values(v):
                    nf = nfp.tile([128, VP, KC, 128], F16, tag="nf")
                    dma(out=nf, in_=nf_d[v])
                    rf = rfp.tile([128, VP, 2, KC, 128], F16, tag="rf")
                    dma(out=rf, in_=rf_d[v])
                    st[("nf", v)] = nf
                    st[("rf", v)] = rf

                def phase_c(g):
                    """weighted-sum matmuls for group g (values prefetched)."""
                    em = st.pop(("em", g))
                    xp = xps.tile([128, 2, KC, G], F32, tag="xp")
                    for vj in range(VPJ):
                        nf = st.pop(("nf", g * VPJ + vj))
                        rf = st.pop(("rf", g * VPJ + vj))
                        for i in range(VP):
                            jj = vj * VP + i
                            for c in range(KC):
                                nc.tensor.matmul(
                                    xp[:, 0, c, jj:jj + 1], nf[:, i, c, :],
                                    em[:, 0, jj:jj + 1],
                                    start=True, stop=True)
                            for c in range(KC):
                                nc.tensor.matmul(
                                    xp[:, 1, c, jj:jj + 1], rf[:, i, 0, c, :],
                                    em[:, 1, jj:jj + 1],
                                    start=True, stop=False)
                                nc.tensor.matmul(
                                    xp[:, 1, c, jj:jj + 1], rf[:, i, 1, c, :],
                                    em[:, 2, jj:jj + 1],
                                    start=False, stop=True)
                    g0 = g * G
                    nc.vector.tensor_copy(XTn[:, :, g0:g0 + G], xp[:, 0])
                    nc.vector.tensor_copy(XTr[:, :, g0:g0 + G], xp[:, 1])

                PVLAG = 4  # value stream trails by 4 score blocks
                for g in range(GROUPS):
                    sc = scps.tile([128, 3, G], F32, tag="sc")
                    for j in range(NPJ):
                        blk = g * NPJ + j
                        phase_a(blk)
                        if blk >= PVLAG:
                            prefetch_values(blk - PVLAG)
                        phase_a_compute(blk, sc, j)
                    if g == GROUPS - 1:
                        for v in range(NVBLK - PVLAG, NVBLK):
                            prefetch_values(v)
                        dma(out=Wng_sb, in_=Wng_d)
                        dma(out=bng_sb, in_=bng_d)
                        dma(out=Wrg_sb, in_=Wrg_d)
                        dma(out=brg_sb, in_=brg_d)
                    if g > 0:
                        phase_c(g - 1)
                    phase_b(g, sc)
                phase_c(GROUPS - 1)

            # ---- normalizers: rS columns [BS, 2] (node, rela) ----
            with (
                tc.tile_pool(name="glue", bufs=1) as gp,
                tc.tile_pool(name="glps", bufs=1, space="PSUM") as gps,
                tc.tile_pool(name="trps", bufs=1, space="PSUM") as tps_p,
            ):
                nc.vector.tensor_add(S_sb[:, 1, :], S_sb[:, 1, :], S_sb[:, 2, :])
                rS = gp.tile([1, 2, BS], F32, tag="rs")
                nc.vector.reciprocal(rS, S_sb[:, 0:2, :])
                rS16 = gp.tile([1, 2, BS], F16, tag="rs16")
                nc.vector.tensor_copy(rS16, rS)
                rSc_ps = gps.tile([BS, 2], F32, tag="rscp")
                for k in range(2):
                    nc.tensor.matmul(rSc_ps[:, k:k + 1], rS16[:, k, :],
                                     ones_col[:1, :1], start=True, stop=True)
                rSc = gp.tile([BS, 2], F32, tag="rsc")
                nc.vector.tensor_copy(rSc, rSc_ps)

                # ---- GLU heads (interleaved: head-2's first matmul group
                # does not depend on node_res, so it overlaps head-1's
                # epilogue) ----
                def pmm(dst, lhs, W_sb, c0):
                    for hh in range(2):
                        for c in range(KC):
                            nc.tensor.matmul(dst[:, hh, :], lhs[:, c, :],
                                             W_sb[:, c0 + c, hh, :],
                                             start=(c == 0), stop=(c == KC - 1))

                p1 = gps.tile([BS, 2, 512], F32, tag="p1")
                p2 = gps.tile([BS, 2, 512], F32, tag="p2")
                q1 = gps.tile([BS, 2, 512], F32, tag="q1")
                pmm(p1, XTn, Wng_sb, 0)            # Xn @ Wng_top
                s1 = gp.tile([BS, 2, 512], F32, tag="ngs1")
                nc.vector.scalar_tensor_tensor(
                    out=s1, in0=p1, scalar=rSc[:, 0:1],
                    in1=bng_sb, op0=ALU.mult, op1=ALU.add)
                pmm(p2, XTr, Wng_sb, KC)           # Xr @ Wng_bot
                pmm(q1, XTr, Wrg_sb, 0)            # Xr @ Wrg_top (head 2)
                t1 = gp.tile([BS, 2, 512], F32, tag="rgs1")
                nc.vector.scalar_tensor_tensor(
                    out=t1, in0=q1, scalar=rSc[:, 1:2],
                    in1=brg_sb, op0=ALU.mult, op1=ALU.add)
                nc.vector.scalar_tensor_tensor(
                    out=s1, in0=p2, scalar=rSc[:, 1:2],
                    in1=s1, op0=ALU.mult, op1=ALU.add)
                sig = gp.tile([BS, 512], F32, tag="ngsig")
                nc.scalar.activation(sig, s1[:, 1, :], AF.Sigmoid)
                nres = gp.tile([BS, 512], F32, tag="ngres")
                nc.vector.tensor_mul(nres, s1[:, 0, :], sig)
                dma(out=nres_d, in_=nres)

                # transpose node_res for the second head
                nres16 = gp.tile([BS, D], F16, tag="n16")
                nc.vector.tensor_copy(nres16, nres)
                nresT = gp.tile([128, KC, BS], F16, tag="nrt")
                for c in range(KC):
                    tps = tps_p.tile([128, BS], F16, tag="tps2")
                    nc.tensor.transpose(tps, nres16[:, c * 128:(c + 1) * 128],
                                        ident[:BS, :BS])
                    nc.vector.tensor_copy(nresT[:, c, :], tps)

                q2 = gps.tile([BS, 2, 512], F32, tag="p1")  # reuse p1's bank
                pmm(q2, nresT, Wrg_sb, KC)         # node_res @ Wrg_bot
                nc.vector.tensor_add(t1, t1, q2)
                sig2 = gp.tile([BS, 512], F32, tag="rgsig")
                nc.scalar.activation(sig2, t1[:, 1, :], AF.Sigmoid)
                rres = gp.tile([BS, 512], F32, tag="rgres")
                nc.vector.tensor_mul(rres, t1[:, 0, :], sig2)
                dma(out=rres_d, in_=rres)

    nc.compile()
    return nc


def make_in_maps(inputs):
    """Shard + lay out full inputs into 8 per-core input dicts (host-side)."""
    f16 = np.float16
    f32 = np.float32

    pnf = np.asarray(inputs["p_node_feats"], dtype=f16)
    nf = np.asarray(inputs["node_feats"], dtype=f16)
    prf = np.asarray(inputs["p_rela_feats"], dtype=f16)
    rf = np.asarray(inputs["rela_feats"], dtype=f16)
    h = np.asarray(inputs["h"], dtype=f16)
    am = np.asarray(inputs["att_masks"], dtype=f16)
    rm = np.asarray(inputs["rela_masks"], dtype=f16)

    def shuf_pnf(x):  # [BS,N,D] -> [NBLK,128,PAIR,KC,N]  (d-partition)
        x = x.reshape(NBLK, PAIR, N, KC, 128)
        return np.ascontiguousarray(x.transpose(0, 4, 1, 3, 2))

    def shuf_prf(x):  # [BS,R,D] -> [NBLK,128,PAIR,KC,R]
        x = x.reshape(NBLK, PAIR, R, KC, 128)
        return np.ascontiguousarray(x.transpose(0, 4, 1, 3, 2))

    def shuf_nf(x):  # [BS,N,D] -> [NVBLK,128,VP,KC,128]  (n-partition)
        x = x.reshape(NVBLK, VP, N, KC, 128)
        return np.ascontiguousarray(x.transpose(0, 2, 1, 3, 4))

    def shuf_rf(x):  # [BS,R,D] -> [NVBLK,128,VP,2,KC,128]
        x = x.reshape(NVBLK, VP, 2, 128, KC, 128)
        return np.ascontiguousarray(x.transpose(0, 3, 1, 2, 4, 5))

    def wcols(w):  # [D] -> [128, KC]
        return np.ascontiguousarray(
            np.asarray(w, dtype=f16).reshape(KC, 128).T)

    Wn = np.ascontiguousarray(
        np.asarray(inputs["W_h2node"], dtype=f16).reshape(KC, 128, D)
        .transpose(1, 0, 2))
    Wr = np.ascontiguousarray(
        np.asarray(inputs["W_h2rela"], dtype=f16).reshape(KC, 128, D)
        .transpose(1, 0, 2))
    Wng = np.ascontiguousarray(
        np.asarray(inputs["W_ng"], dtype=f16).reshape(KC2, 128, 2, 512)
        .transpose(1, 0, 2, 3))
    Wrg = np.ascontiguousarray(
        np.asarray(inputs["W_rg"], dtype=f16).reshape(KC2, 128, 2, 512)
        .transpose(1, 0, 2, 3))
    bng = np.ascontiguousarray(np.broadcast_to(
        np.asarray(inputs["b_ng"], dtype=f32), (BS, 2 * D)).reshape(BS, 2, 512))
    brg = np.ascontiguousarray(np.broadcast_to(
        np.asarray(inputs["b_rg"], dtype=f32), (BS, 2 * D)).reshape(BS, 2, 512))

    shared = {
        "w_h2node": Wn, "w_h2rela": Wr,
        "b_h2node": np.ascontiguousarray(
            np.asarray(inputs["b_h2node"], dtype=f32).reshape(KC, 128).T),
        "b_h2rela": np.ascontiguousarray(
            np.asarray(inputs["b_h2rela"], dtype=f32).reshape(KC, 128).T),
        "w1c": wcols(inputs["w_alpha1"]),
        "w2c": wcols(inputs["w_alpha2"]),
        "w_ng": Wng, "w_rg": Wrg, "bias_ng": bng, "bias_rg": brg,
        "ident": np.eye(128, dtype=f16),
        "ones_col": np.ones((128, 1), dtype=f16),
        "ones_row": np.ones((1, 128), dtype=f16),
    }
    in_maps = []
    for cix in range(NCORES):
        s = slice(cix * BS, (cix + 1) * BS)
        mT = np.empty((128, 3, BS), dtype=f16)
        mT[:, 0, :] = am[s].T
        mT[:, 1, :] = rm[s, :128].T
        mT[:, 2, :] = rm[s, 128:].T
        in_maps.append({
            "h": np.ascontiguousarray(h[s]),
            "pnf": shuf_pnf(pnf[s]), "prf": shuf_prf(prf[s]),
            "nf": shuf_nf(nf[s]), "rf": shuf_rf(rf[s]),
            "mT": np.ascontiguousarray(mT), **shared,
        })
    return in_maps


_NC_CACHE = None
LAST_RESULTS = None  # BassKernelResults of the most recent kernel() call


def kernel(**inputs):
    global _NC_CACHE, LAST_RESULTS
    if _NC_CACHE is None:
        _NC_CACHE = build_program()
    nc = _NC_CACHE
    in_maps = make_in_maps(inputs)
    import os
    trace = os.environ.get("BASS_KERNEL_TRACE", "0") == "1"
    res = run_bass_kernel_spmd(nc, in_maps, core_ids=list(range(NCORES)),
                               trace=trace)
    LAST_RESULTS = res
    node_res = np.concatenate([r["node_res"] for r in res.results], axis=0)
    rela_res = np.concatenate([r["rela_res"] for r in res.results], axis=0)
    return node_res, rela_res


# revision 36
# speedup vs baseline: 1.0855x; 1.0855x over previous
"""Trainium2 Bass kernel for nn_Attention_60885456388891 (gnn_message_passing).

Computation (per batch b):
  node_h = h @ W_h2node + b_h2node
  score_n[n] = sum_d tanh(p_node_feats[b,n,d] + node_h[b,d]) * w_alpha1[d]
  node_w = renorm(softmax(score_n) * att_masks)
  node_res_ = sum_n node_w[n] * node_feats[b,n,:]
  (same for relations)
  node_res = glu(cat(node_res_, rela_res_) @ W_ng + b_ng)
  rela_res = glu(cat(rela_res_, node_res) @ W_rg + b_rg)

Strategy: pure data-parallel over batch B=512 across 8 cores (64 batches/core),
all features downcast to fp16 on the host (halves HBM traffic; rel-err ~1e-3
vs the 2e-2 gate).

Per-core pipeline (v4 design):
  - pnf/prf streamed in d-on-partitions layout: broadcast-add of node_h/rela_h
    becomes a per-partition-scalar DVE add (fp16 4x-ish mode), tanh batches
    into large ACT calls.
  - scores via tiny tanh-stationary PE matmuls (FWL fast weight load):
    lhsT = tanh chunk [128d, 128n], rhs = w_alpha chunk col -> score columns
    accumulate in PSUM.  |score| <= sum|w_alpha| ~ 8, so exp() needs no max
    subtraction; masked-exp columns are used UNNORMALIZED as weights, and the
    1/sum(EM) normalizer is folded into the GLU epilogue as a per-row scale.
  - phase C: nf/rf chunk stationary [128n, 128d] x EM column -> X^T columns
    accumulate directly in the k-chunked layout the GLU matmuls consume (no
    row staging, no transposes).
  - GLU: out = (Xn_u @ Wtop) * rSn + (Xr_u @ Wbot) * rSr + bias_bcast,
    a*sigmoid(g).
"""

import numpy as np

import concourse.bass as bass
import concourse.bacc as bacc
import concourse.mybir as mybir
import concourse.tile as tile
from concourse.bass_utils import run_bass_kernel_spmd

# Problem dims (hardcoded per contract)
B, N, R, D = 512, 128, 256, 512
NCORES = 8
BS = B // NCORES          # 64 batches per core
PAIR = 4                  # batches per stream DMA block
NBLK = BS // PAIR         # 16 blocks
G = 16                    # batches per softmax/psum group
GROUPS = BS // G          # 4 groups
KC = D // 128             # 4 k-chunks of 128
KC2 = 2 * D // 128        # 8 k-chunks for the 1024-wide GLU matmuls
VP = PAIR                 # batches per value-stream DMA block
NVBLK = BS // VP          # value blocks
VPJ = G // VP             # value blocks per group

F32 = mybir.dt.float32
F16 = mybir.dt.float16
AF = mybir.ActivationFunctionType
ALU = mybir.AluOpType
AX = mybir.AxisListType


def build_program():
    nc = bacc.Bacc("TRN2", target_bir_lowering=False, debug=False)

    def din(name, shape, dt=F16):
        return nc.dram_tensor(name, shape, dt, kind="ExternalInput").ap()

    h_d = din("h", [BS, D])
    pnf_d = din("pnf", [NBLK, 128, PAIR, KC, N])        # d-partition args
    prf_d = din("prf", [NBLK, 128, PAIR, KC, R])
    nf_d = din("nf", [NVBLK, 128, VP, KC, 128])         # n-partition values
    rf_d = din("rf", [NVBLK, 128, VP, 2, KC, 128])
    mT_d = din("mT", [128, 3, BS])                      # masks, transposed
    Wn_d = din("w_h2node", [128, KC, D])
    bn_d = din("b_h2node", [128, KC], F32)
    Wr_d = din("w_h2rela", [128, KC, D])
    br_d = din("b_h2rela", [128, KC], F32)
    w1_d = din("w1c", [128, KC])                        # w_alpha1 as columns
    w2_d = din("w2c", [128, KC])
    Wng_d = din("w_ng", [128, KC2, 2, 512])
    Wrg_d = din("w_rg", [128, KC2, 2, 512])
    bng_d = din("bias_ng", [BS, 2, 512], F32)           # host-broadcast bias
    brg_d = din("bias_rg", [BS, 2, 512], F32)
    id_d = din("ident", [128, 128])                     # f16 identity
    ones_d = din("ones_col", [128, 1])                  # f16 ones column
    onesr_d = din("ones_row", [1, 128])                 # f16 ones row

    nres_d = nc.dram_tensor("node_res", [BS, D], F32, kind="ExternalOutput").ap()
    rres_d = nc.dram_tensor("rela_res", [BS, D], F32, kind="ExternalOutput").ap()

    dma = nc.sync.dma_start
    dma_s = nc.gpsimd.dma_start

    with tile.TileContext(nc) as tc:
        with (
            tc.tile_pool(name="const", bufs=1) as cp,
        ):
            # ---- persistent constants ----
            ident = cp.tile([128, 128], F16)
            dma(out=ident, in_=id_d)
            ones_col = cp.tile([128, 1], F16)
            dma(out=ones_col, in_=ones_d)
            ones_row = cp.tile([1, 128], F16)
            dma(out=ones_row, in_=onesr_d)
            w1c = cp.tile([128, KC], F16)
            dma(out=w1c, in_=w1_d)
            w2c = cp.tile([128, KC], F16)
            dma(out=w2c, in_=w2_d)
            mT = cp.tile([128, 3, BS], F16)
            dma(out=mT, in_=mT_d)

            # persistent outputs of phase C / B
            XTn = cp.tile([128, KC, BS], F16, tag="xtn")   # unnormalized Xn^T
            XTr = cp.tile([128, KC, BS], F16, tag="xtr")
            S_sb = cp.tile([1, 3, BS], F32, tag="ssb")     # EM column sums
            nhT = cp.tile([128, KC, BS], F32, tag="nht")   # bias columns
            rhT = cp.tile([128, KC, BS], F32, tag="rht")
            # GLU weights/biases (DMA'd during the last group's streaming)
            Wng_sb = cp.tile([128, KC2, 2, 512], F16, tag="wng")
            Wrg_sb = cp.tile([128, KC2, 2, 512], F16, tag="wrg")
            bng_sb = cp.tile([BS, 2, 512], F32, tag="bng")
            brg_sb = cp.tile([BS, 2, 512], F32, tag="brg")

            # ---- prologue: nhT/rhT bias columns = (h @ W + b)^T, computed
            # directly in transposed chunk layout (no row round-trip) ----
            with (
                tc.tile_pool(name="prol", bufs=1) as pp,
                tc.tile_pool(name="prps", bufs=2, space="PSUM") as pps,
            ):
                h_sb = pp.tile([BS, D], F16, tag="h")
                dma(out=h_sb, in_=h_d)
                Wn_sb = pp.tile([128, KC, D], F16, tag="wn")
                dma(out=Wn_sb, in_=Wn_d)
                Wr_sb = pp.tile([128, KC, D], F16, tag="wr")
                dma(out=Wr_sb, in_=Wr_d)
                bn_sb = pp.tile([128, KC], F32, tag="bn")
                dma(out=bn_sb, in_=bn_d)
                br_sb = pp.tile([128, KC], F32, tag="br")
                dma(out=br_sb, in_=br_d)

                hT = pp.tile([128, KC, BS], F16, tag="ht")
                for c in range(KC):
                    tps = pps.tile([128, BS], F16, tag="tps")
                    nc.tensor.transpose(tps, h_sb[:, c * 128:(c + 1) * 128],
                                        ident[:BS, :BS])
                    nc.vector.tensor_copy(hT[:, c, :], tps)
                for W_sb, b_sb, dstT in ((Wn_sb, bn_sb, nhT),
                                         (Wr_sb, br_sb, rhT)):
                    for c in range(KC):
                        ps = pps.tile([128, BS], F32, tag="nhcps")
                        for k in range(KC):
                            nc.tensor.matmul(
                                ps, W_sb[:, k, c * 128:(c + 1) * 128],
                                hT[:, k, :],
                                start=(k == 0), stop=(k == KC - 1))
                        nc.vector.tensor_scalar_add(dstT[:, c, :], ps,
                                                    b_sb[:, c:c + 1])

            # ---- main loop: per-PAIR software pipeline over 16 slots ----
            # Slot k: [pnf/prf DMA (k)] [exp/mask/S (k-1)] [phase-C mm (k-2)]
            #         [adds+tanh (k)] [score mm (k)] [nf/rf prefetch DMA (k)]
            with (
                tc.tile_pool(name="pnfp", bufs=5) as pnfp,
                tc.tile_pool(name="prfp", bufs=4) as prfp,
                tc.tile_pool(name="nfp", bufs=6) as nfp,
                tc.tile_pool(name="rfp", bufs=6) as rfp,
                tc.tile_pool(name="emp", bufs=2) as emp,
                tc.tile_pool(name="scps", bufs=2, space="PSUM") as scps,
                tc.tile_pool(name="xps", bufs=2, space="PSUM") as xps,
                tc.tile_pool(name="sps", bufs=2, space="PSUM") as sps,
            ):
                NPJ = G // PAIR       # pair slots per group
                NSLOT = NBLK          # total slots
                LAG = 2               # phase-C runs LAG slots behind phase-A
                st = {}               # per-slot live tiles

                def phase_a(k):
                    pnf = pnfp.tile([128, PAIR, KC, N], F16, tag="pnf")
                    dma(out=pnf, in_=pnf_d[k])
                    prf = prfp.tile([128, PAIR, KC, R], F16, tag="prf")
                    dma(out=prf, in_=prf_d[k])
                    st[("pnf", k)] = pnf
                    st[("prf", k)] = prf

                def phase_a_compute(k, sc, j):
                    pnf = st.pop(("pnf", k))
                    prf = st.pop(("prf", k))
                    for i in range(PAIR):
                        b = k * PAIR + i
                        for c in range(KC):
                            nc.vector.tensor_scalar_add(
                                pnf[:, i, c, :], pnf[:, i, c, :],
                                nhT[:, c, b:b + 1])
                            nc.vector.tensor_scalar_add(
                                prf[:, i, c, :], prf[:, i, c, :],
                                rhT[:, c, b:b + 1])
                    nc.scalar.activation(pnf, pnf, AF.Tanh)
                    nc.scalar.activation(prf, prf, AF.Tanh)
                    for i in range(PAIR):
                        jj = j * PAIR + i
                        for c in range(KC):
                            nc.tensor.matmul(
                                sc[:, 0, jj:jj + 1], pnf[:, i, c, :],
                                w1c[:, c:c + 1],
                                start=(c == 0), stop=(c == KC - 1))
                        for c in range(KC):
                            nc.tensor.matmul(
                                sc[:, 1, jj:jj + 1], prf[:, i, c, :128],
                                w2c[:, c:c + 1],
                                start=(c == 0), stop=(c == KC - 1))
                        for c in range(KC):
                            nc.tensor.matmul(
                                sc[:, 2, jj:jj + 1], prf[:, i, c, 128:],
                                w2c[:, c:c + 1],
                                start=(c == 0), stop=(c == KC - 1))

                def phase_b(g, sc):
                    """masked exp + column sums for group g."""
                    g0 = g * G
                    em = emp.tile([128, 3, G], F16, tag="em")
                    st[("em", g)] = em
                    nc.scalar.activation(em, sc, AF.Exp)
                    nc.vector.tensor_mul(em, em, mT[:, :, g0:g0 + G])
                    s_ps = sps.tile([1, 3, G], F32, tag="s")
                    nc.tensor.matmul(s_ps, ones_col, em, start=True, stop=True)
                    nc.vector.tensor_copy(S_sb[:, :, g0:g0 + G], s_ps)

                def prefetch_values(v):
                    nf = nfp.tile([128, VP, KC, 128], F16, tag="nf")
                    dma(out=nf, in_=nf_d[v])
                    rf = rfp.tile([128, VP, 2, KC, 128], F16, tag="rf")
                    dma(out=rf, in_=rf_d[v])
                    st[("nf", v)] = nf
                    st[("rf", v)] = rf

                def phase_c(g):
                    """weighted-sum matmuls for group g (values prefetched)."""
                    em = st.pop(("em", g))
                    xp = xps.tile([128, 2, KC, G], F32, tag="xp")
                    for vj in range(VPJ):
                        nf = st.pop(("nf", g * VPJ + vj))
                        rf = st.pop(("rf", g * VPJ + vj))
                        for i in range(VP):
                            jj = vj * VP + i
                            for c in range(KC):
                                nc.tensor.matmul(
                                    xp[:, 0, c, jj:jj + 1], nf[:, i, c, :],
                                    em[:, 0, jj:jj + 1],
                                    start=True, stop=True)
                            for c in range(KC):
                                nc.tensor.matmul(
                                    xp[:, 1, c, jj:jj + 1], rf[:, i, 0, c, :],
                                    em[:, 1, jj:jj + 1],
                                    start=True, stop=False)
                                nc.tensor.matmul(
                                    xp[:, 1, c, jj:jj + 1], rf[:, i, 1, c, :],
                                    em[:, 2, jj:jj + 1],
                                    start=False, stop=True)
                    g0 = g * G
                    nc.vector.tensor_copy(XTn[:, :, g0:g0 + G], xp[:, 0])
                    nc.vector.tensor_copy(XTr[:, :, g0:g0 + G], xp[:, 1])

                PVLAG = 4  # value stream trails by 4 score blocks
                for g in range(GROUPS):
                    sc = scps.tile([128, 3, G], F32, tag="sc")
                    for j in range(NPJ):
                        blk = g * NPJ + j
                        phase_a(blk)
                        if blk >= PVLAG:
                            prefetch_values(blk - PVLAG)
                        phase_a_compute(blk, sc, j)
                    if g == GROUPS - 1:
                        for v in range(NVBLK - PVLAG, NVBLK):
                            prefetch_values(v)
                        dma(out=Wng_sb, in_=Wng_d)
                        dma(out=bng_sb, in_=bng_d)
                        dma(out=Wrg_sb, in_=Wrg_d)
                        dma(out=brg_sb, in_=brg_d)
                    if g > 0:
                        phase_c(g - 1)
                    phase_b(g, sc)
                phase_c(GROUPS - 1)

            # ---- normalizers: rS columns [BS, 2] (node, rela) ----
            with (
                tc.tile_pool(name="glue", bufs=1) as gp,
                tc.tile_pool(name="glps", bufs=1, space="PSUM") as gps,
                tc.tile_pool(name="trps", bufs=1, space="PSUM") as tps_p,
            ):
                nc.vector.tensor_add(S_sb[:, 1, :], S_sb[:, 1, :], S_sb[:, 2, :])
                rS = gp.tile([1, 2, BS], F32, tag="rs")
                nc.vector.reciprocal(rS, S_sb[:, 0:2, :])
                rS16 = gp.tile([1, 2, BS], F16, tag="rs16")
                nc.vector.tensor_copy(rS16, rS)
                rSc_ps = gps.tile([BS, 2], F32, tag="rscp")
                for k in range(2):
                    nc.tensor.matmul(rSc_ps[:, k:k + 1], rS16[:, k, :],
                                     ones_col[:1, :1], start=True, stop=True)
                rSc = gp.tile([BS, 2], F32, tag="rsc")
                nc.vector.tensor_copy(rSc, rSc_ps)

                # ---- GLU heads (interleaved: head-2's first matmul group
                # does not depend on node_res, so it overlaps head-1's
                # epilogue) ----
                def pmm(dst, lhs, W_sb, c0):
                    for hh in range(2):
                        for c in range(KC):
                            nc.tensor.matmul(dst[:, hh, :], lhs[:, c, :],
                                             W_sb[:, c0 + c, hh, :],
                                             start=(c == 0), stop=(c == KC - 1))

                p1 = gps.tile([BS, 2, 512], F32, tag="p1")
                p2 = gps.tile([BS, 2, 512], F32, tag="p2")
                q1 = gps.tile([BS, 2, 512], F32, tag="q1")
                pmm(p1, XTn, Wng_sb, 0)            # Xn @ Wng_top
                s1 = gp.tile([BS, 2, 512], F32, tag="ngs1")
                nc.vector.scalar_tensor_tensor(
                    out=s1, in0=p1, scalar=rSc[:, 0:1],
                    in1=bng_sb, op0=ALU.mult, op1=ALU.add)
                pmm(p2, XTr, Wng_sb, KC)           # Xr @ Wng_bot
                pmm(q1, XTr, Wrg_sb, 0)            # Xr @ Wrg_top (head 2)
                t1 = gp.tile([BS, 2, 512], F32, tag="rgs1")
                nc.vector.scalar_tensor_tensor(
                    out=t1, in0=q1, scalar=rSc[:, 1:2],
                    in1=brg_sb, op0=ALU.mult, op1=ALU.add)
                nc.vector.scalar_tensor_tensor(
                    out=s1, in0=p2, scalar=rSc[:, 1:2],
                    in1=s1, op0=ALU.mult, op1=ALU.add)
                sig = gp.tile([BS, 512], F32, tag="ngsig")
                nc.scalar.activation(sig, s1[:, 1, :], AF.Sigmoid)
                nres = gp.tile([BS, 512], F32, tag="ngres")
                nc.vector.tensor_mul(nres, s1[:, 0, :], sig)
                dma(out=nres_d, in_=nres)

                # transpose node_res for the second head
                nres16 = gp.tile([BS, D], F16, tag="n16")
                nc.vector.tensor_copy(nres16, nres)
                nresT = gp.tile([128, KC, BS], F16, tag="nrt")
                for c in range(KC):
                    tps = tps_p.tile([128, BS], F16, tag="tps2")
                    nc.tensor.transpose(tps, nres16[:, c * 128:(c + 1) * 128],
                                        ident[:BS, :BS])
                    nc.vector.tensor_copy(nresT[:, c, :], tps)

                q2 = gps.tile([BS, 2, 512], F32, tag="p1")  # reuse p1's bank
                pmm(q2, nresT, Wrg_sb, KC)         # node_res @ Wrg_bot
                nc.vector.tensor_add(t1, t1, q2)
                sig2 = gp.tile([BS, 512], F32, tag="rgsig")
                nc.scalar.activation(sig2, t1[:, 1, :], AF.Sigmoid)
                rres = gp.tile([BS, 512], F32, tag="rgres")
                nc.vector.tensor_mul(rres, t1[:, 0, :], sig2)
                dma(out=rres_d, in_=rres)

    nc.compile()
    return nc


def make_in_maps(inputs):
    """Shard + lay out full inputs into 8 per-core input dicts (host-side)."""
    f16 = np.float16
    f32 = np.float32

    pnf = np.asarray(inputs["p_node_feats"], dtype=f16)
    nf = np.asarray(inputs["node_feats"], dtype=f16)
    prf = np.asarray(inputs["p_rela_feats"], dtype=f16)
    rf = np.asarray(inputs["rela_feats"], dtype=f16)
    h = np.asarray(inputs["h"], dtype=f16)
    am = np.asarray(inputs["att_masks"], dtype=f16)
    rm = np.asarray(inputs["rela_masks"], dtype=f16)

    def shuf_pnf(x):  # [BS,N,D] -> [NBLK,128,PAIR,KC,N]  (d-partition)
        x = x.reshape(NBLK, PAIR, N, KC, 128)
        return np.ascontiguousarray(x.transpose(0, 4, 1, 3, 2))

    def shuf_prf(x):  # [BS,R,D] -> [NBLK,128,PAIR,KC,R]
        x = x.reshape(NBLK, PAIR, R, KC, 128)
        return np.ascontiguousarray(x.transpose(0, 4, 1, 3, 2))

    def shuf_nf(x):  # [BS,N,D] -> [NVBLK,128,VP,KC,128]  (n-partition)
        x = x.reshape(NVBLK, VP, N, KC, 128)
        return np.ascontiguousarray(x.transpose(0, 2, 1, 3, 4))

    def shuf_rf(x):  # [BS,R,D] -> [NVBLK,128,VP,2,KC,128]
        x = x.reshape(NVBLK, VP, 2, 128, KC, 128)
        return np.ascontiguousarray(x.transpose(0, 3, 1, 2, 4, 5))

    def wcols(w):  # [D] -> [128, KC]
        return np.ascontiguousarray(
            np.asarray(w, dtype=f16).reshape(KC, 128).T)

    Wn = np.ascontiguousarray(
        np.asarray(inputs["W_h2node"], dtype=f16).reshape(KC, 128, D)
        .transpose(1, 0, 2))
    Wr = np.ascontiguousarray(
        np.asarray(inputs["W_h2rela"], dtype=f16).reshape(KC, 128, D)
        .transpose(1, 0, 2))
    Wng = np.ascontiguousarray(
        np.asarray(inputs["W_ng"], dtype=f16).reshape(KC2, 128, 2, 512)
        .transpose(1, 0, 2, 3))
    Wrg = np.ascontiguousarray(
        np.asarray(inputs["W_rg"], dtype=f16).reshape(KC2, 128, 2, 512)
        .transpose(1, 0, 2, 3))
    bng = np.ascontiguousarray(np.broadcast_to(
        np.asarray(inputs["b_ng"], dtype=f32), (BS, 2 * D)).reshape(BS, 2, 512))
    brg = np.ascontiguousarray(np.broadcast_to(
        np.asarray(inputs["b_rg"], dtype=f32), (BS, 2 * D)).reshape(BS, 2, 512))

    shared = {
        "w_h2node": Wn, "w_h2rela": Wr,
        "b_h2node": np.ascontiguousarray(
            np.asarray(inputs["b_h2node"], dtype=f32).reshape(KC, 128).T),
        "b_h2rela": np.ascontiguousarray(
            np.asarray(inputs["b_h2rela"], dtype=f32).reshape(KC, 128).T),
        "w1c": wcols(inputs["w_alpha1"]),
        "w2c": wcols(inputs["w_alpha2"]),
        "w_ng": Wng, "w_rg": Wrg, "bias_ng": bng, "bias_rg": brg,
        "ident": np.eye(128, dtype=f16),
        "ones_col": np.ones((128, 1), dtype=f16),
        "ones_row": np.ones((1, 128), dtype=f16),
    }
    in_maps = []
    for cix in range(NCORES):
        s = slice(cix * BS, (cix + 1) * BS)
        mT = np.empty((128, 3, BS), dtype=f16)
        mT[:, 0, :] = am[s].T
        mT[:, 1, :] = rm[s, :128].T
        mT[:, 2, :] = rm[s, 128:].T
        in_maps.append({
            "h": np.ascontiguousarray(h[s]),
            "pnf": shuf_pnf(pnf[s]), "prf": shuf_prf(prf[s]),
            "nf": shuf_nf(nf[s]), "rf": shuf_rf(rf[s]),
            "mT": np.ascontiguousarray(mT), **shared,
        })
    return in_maps


_NC_CACHE = None
LAST_RESULTS = None  # BassKernelResults of the most recent kernel() call


def kernel(**inputs):
    global _NC_CACHE, LAST_RESULTS
    if _NC_CACHE is None:
        _NC_CACHE = build_program()
    nc = _NC_CACHE
    in_maps = make_in_maps(inputs)
    import os
    trace = os.environ.get("BASS_KERNEL_TRACE", "0") == "1"
    res = run_bass_kernel_spmd(nc, in_maps, core_ids=list(range(NCORES)),
                               trace=trace)
    LAST_RESULTS = res
    node_res = np.concatenate([r["node_res"] for r in res.results], axis=0)
    rela_res = np.concatenate([r["rela_res"] for r in res.results], axis=0)
    return node_res, rela_res


# revision 37
# speedup vs baseline: 1.1281x; 1.0393x over previous
"""Trainium2 Bass kernel for nn_Attention_60885456388891 (gnn_message_passing).

Computation (per batch b):
  node_h = h @ W_h2node + b_h2node
  score_n[n] = sum_d tanh(p_node_feats[b,n,d] + node_h[b,d]) * w_alpha1[d]
  node_w = renorm(softmax(score_n) * att_masks)
  node_res_ = sum_n node_w[n] * node_feats[b,n,:]
  (same for relations)
  node_res = glu(cat(node_res_, rela_res_) @ W_ng + b_ng)
  rela_res = glu(cat(rela_res_, node_res) @ W_rg + b_rg)

Strategy: pure data-parallel over batch B=512 across 8 cores (64 batches/core),
all features downcast to fp16 on the host (halves HBM traffic; rel-err ~1e-3
vs the 2e-2 gate).

Per-core pipeline (v4 design):
  - pnf/prf streamed in d-on-partitions layout: broadcast-add of node_h/rela_h
    becomes a per-partition-scalar DVE add (fp16 4x-ish mode), tanh batches
    into large ACT calls.
  - scores via tiny tanh-stationary PE matmuls (FWL fast weight load):
    lhsT = tanh chunk [128d, 128n], rhs = w_alpha chunk col -> score columns
    accumulate in PSUM.  |score| <= sum|w_alpha| ~ 8, so exp() needs no max
    subtraction; masked-exp columns are used UNNORMALIZED as weights, and the
    1/sum(EM) normalizer is folded into the GLU epilogue as a per-row scale.
  - phase C: nf/rf chunk stationary [128n, 128d] x EM column -> X^T columns
    accumulate directly in the k-chunked layout the GLU matmuls consume (no
    row staging, no transposes).
  - GLU: out = (Xn_u @ Wtop) * rSn + (Xr_u @ Wbot) * rSr + bias_bcast,
    a*sigmoid(g).
"""

import numpy as np

import concourse.bass as bass
import concourse.bacc as bacc
import concourse.mybir as mybir
import concourse.tile as tile
from concourse.bass_utils import run_bass_kernel_spmd

# Problem dims (hardcoded per contract)
B, N, R, D = 512, 128, 256, 512
NCORES = 8
BS = B // NCORES          # 64 batches per core
PAIR = 4                  # batches per stream DMA block
NBLK = BS // PAIR         # 16 blocks
G = 16                    # batches per softmax/psum group
GROUPS = BS // G          # 4 groups
KC = D // 128             # 4 k-chunks of 128
KC2 = 2 * D // 128        # 8 k-chunks for the 1024-wide GLU matmuls
VP = PAIR                 # batches per value-stream DMA block
NVBLK = BS // VP          # value blocks
VPJ = G // VP             # value blocks per group

F32 = mybir.dt.float32
F16 = mybir.dt.float16
AF = mybir.ActivationFunctionType
ALU = mybir.AluOpType
AX = mybir.AxisListType


def build_program():
    nc = bacc.Bacc("TRN2", target_bir_lowering=False, debug=False)

    def din(name, shape, dt=F16):
        return nc.dram_tensor(name, shape, dt, kind="ExternalInput").ap()

    h_d = din("h", [BS, D])
    pnf_d = din("pnf", [NBLK, 128, PAIR, KC, N])        # d-partition args
    prf_d = din("prf", [NBLK, 128, PAIR, KC, R])
    nf_d = din("nf", [NVBLK, 128, VP, KC, 128])         # n-partition values
    rf_d = din("rf", [NVBLK, 128, VP, 2, KC, 128])
    mT_d = din("mT", [128, 3, BS])                      # masks, transposed
    Wn_d = din("w_h2node", [128, KC, D])
    bn_d = din("b_h2node", [128, KC], F32)
    Wr_d = din("w_h2rela", [128, KC, D])
    br_d = din("b_h2rela", [128, KC], F32)
    w1_d = din("w1c", [128, KC])                        # w_alpha1 as columns
    w2_d = din("w2c", [128, KC])
    Wng_d = din("w_ng", [128, KC2, 2, 512])
    Wrg_d = din("w_rg", [128, KC2, 2, 512])
    bng_d = din("bias_ng", [BS, 2, 512], F32)           # host-broadcast bias
    brg_d = din("bias_rg", [BS, 2, 512], F32)
    id_d = din("ident", [128, 128])                     # f16 identity
    ones_d = din("ones_col", [128, 1])                  # f16 ones column
    onesr_d = din("ones_row", [1, 128])                 # f16 ones row

    nres_d = nc.dram_tensor("node_res", [BS, D], F32, kind="ExternalOutput").ap()
    rres_d = nc.dram_tensor("rela_res", [BS, D], F32, kind="ExternalOutput").ap()

    dma = nc.sync.dma_start
    dma_s = nc.gpsimd.dma_start

    with tile.TileContext(nc) as tc:
        with (
            tc.tile_pool(name="const", bufs=1) as cp,
        ):
            # ---- persistent constants ----
            ident = cp.tile([128, 128], F16)
            dma(out=ident, in_=id_d)
            ones_col = cp.tile([128, 1], F16)
            dma(out=ones_col, in_=ones_d)
            ones_row = cp.tile([1, 128], F16)
            dma(out=ones_row, in_=onesr_d)
            w1c = cp.tile([128, KC], F16)
            dma(out=w1c, in_=w1_d)
            w2c = cp.tile([128, KC], F16)
            dma(out=w2c, in_=w2_d)
            mT = cp.tile([128, 3, BS], F16)
            dma(out=mT, in_=mT_d)

            # persistent outputs of phase C / B
            XTn = cp.tile([128, KC, BS], F16, tag="xtn")   # unnormalized Xn^T
            XTr = cp.tile([128, KC, BS], F16, tag="xtr")
            S_sb = cp.tile([1, 3, BS], F32, tag="ssb")     # EM column sums
            nhT = cp.tile([128, KC, BS], F32, tag="nht")   # bias columns
            rhT = cp.tile([128, KC, BS], F32, tag="rht")
            # GLU weights/biases (DMA'd during the last group's streaming)
            Wng_sb = cp.tile([128, KC2, 2, 512], F16, tag="wng")
            Wrg_sb = cp.tile([128, KC2, 2, 512], F16, tag="wrg")
            bng_sb = cp.tile([BS, 2, 512], F32, tag="bng")
            brg_sb = cp.tile([BS, 2, 512], F32, tag="brg")

            # ---- prologue: nhT/rhT bias columns = (h @ W + b)^T, computed
            # directly in transposed chunk layout (no row round-trip) ----
            with (
                tc.tile_pool(name="prol", bufs=1) as pp,
                tc.tile_pool(name="prps", bufs=2, space="PSUM") as pps,
            ):
                h_sb = pp.tile([BS, D], F16, tag="h")
                dma(out=h_sb, in_=h_d)
                Wn_sb = pp.tile([128, KC, D], F16, tag="wn")
                dma(out=Wn_sb, in_=Wn_d)
                Wr_sb = pp.tile([128, KC, D], F16, tag="wr")
                dma(out=Wr_sb, in_=Wr_d)
                bn_sb = pp.tile([128, KC], F32, tag="bn")
                dma(out=bn_sb, in_=bn_d)
                br_sb = pp.tile([128, KC], F32, tag="br")
                dma(out=br_sb, in_=br_d)

                hT = pp.tile([128, KC, BS], F16, tag="ht")
                for c in range(KC):
                    tps = pps.tile([128, BS], F16, tag="tps")
                    nc.tensor.transpose(tps, h_sb[:, c * 128:(c + 1) * 128],
                                        ident[:BS, :BS])
                    nc.vector.tensor_copy(hT[:, c, :], tps)
                for W_sb, b_sb, dstT in ((Wn_sb, bn_sb, nhT),
                                         (Wr_sb, br_sb, rhT)):
                    for c in range(KC):
                        ps = pps.tile([128, BS], F32, tag="nhcps")
                        for k in range(KC):
                            nc.tensor.matmul(
                                ps, W_sb[:, k, c * 128:(c + 1) * 128],
                                hT[:, k, :],
                                start=(k == 0), stop=(k == KC - 1))
                        nc.vector.tensor_scalar_add(dstT[:, c, :], ps,
                                                    b_sb[:, c:c + 1])

            # ---- main loop: per-PAIR software pipeline over 16 slots ----
            # Slot k: [pnf/prf DMA (k)] [exp/mask/S (k-1)] [phase-C mm (k-2)]
            #         [adds+tanh (k)] [score mm (k)] [nf/rf prefetch DMA (k)]
            with (
                tc.tile_pool(name="pnfp", bufs=6) as pnfp,
                tc.tile_pool(name="prfp", bufs=6) as prfp,
                tc.tile_pool(name="nfp", bufs=6) as nfp,
                tc.tile_pool(name="rfp", bufs=6) as rfp,
                tc.tile_pool(name="emp", bufs=2) as emp,
                tc.tile_pool(name="scps", bufs=2, space="PSUM") as scps,
                tc.tile_pool(name="xps", bufs=2, space="PSUM") as xps,
                tc.tile_pool(name="sps", bufs=2, space="PSUM") as sps,
            ):
                NPJ = G // PAIR       # pair slots per group
                NSLOT = NBLK          # total slots
                LAG = 2               # phase-C runs LAG slots behind phase-A
                st = {}               # per-slot live tiles

                def phase_a(k):
                    pnf = pnfp.tile([128, PAIR, KC, N], F16, tag="pnf")
                    dma(out=pnf, in_=pnf_d[k])
                    prf = prfp.tile([128, PAIR, KC, R], F16, tag="prf")
                    dma(out=prf, in_=prf_d[k])
                    st[("pnf", k)] = pnf
                    st[("prf", k)] = prf

                def phase_a_compute(k, sc, j):
                    pnf = st.pop(("pnf", k))
                    prf = st.pop(("prf", k))
                    for i in range(PAIR):
                        b = k * PAIR + i
                        for c in range(KC):
                            nc.vector.tensor_scalar_add(
                                pnf[:, i, c, :], pnf[:, i, c, :],
                                nhT[:, c, b:b + 1])
                            nc.vector.tensor_scalar_add(
                                prf[:, i, c, :], prf[:, i, c, :],
                                rhT[:, c, b:b + 1])
                    nc.scalar.activation(pnf, pnf, AF.Tanh)
                    nc.scalar.activation(prf, prf, AF.Tanh)
                    for i in range(PAIR):
                        jj = j * PAIR + i
                        for c in range(KC):
                            nc.tensor.matmul(
                                sc[:, 0, jj:jj + 1], pnf[:, i, c, :],
                                w1c[:, c:c + 1],
                                start=(c == 0), stop=(c == KC - 1))
                        for c in range(KC):
                            nc.tensor.matmul(
                                sc[:, 1, jj:jj + 1], prf[:, i, c, :128],
                                w2c[:, c:c + 1],
                                start=(c == 0), stop=(c == KC - 1))
                        for c in range(KC):
                            nc.tensor.matmul(
                                sc[:, 2, jj:jj + 1], prf[:, i, c, 128:],
                                w2c[:, c:c + 1],
                                start=(c == 0), stop=(c == KC - 1))

                def phase_b(g, sc):
                    """masked exp + column sums for group g."""
                    g0 = g * G
                    em = emp.tile([128, 3, G], F16, tag="em")
                    st[("em", g)] = em
                    nc.scalar.activation(em, sc, AF.Exp)
                    nc.vector.tensor_mul(em, em, mT[:, :, g0:g0 + G])
                    s_ps = sps.tile([1, 3, G], F32, tag="s")
                    nc.tensor.matmul(s_ps, ones_col, em, start=True, stop=True)
                    nc.vector.tensor_copy(S_sb[:, :, g0:g0 + G], s_ps)

                def prefetch_values(v):
                    nf = nfp.tile([128, VP, KC, 128], F16, tag="nf")
                    dma(out=nf, in_=nf_d[v])
                    rf = rfp.tile([128, VP, 2, KC, 128], F16, tag="rf")
                    dma(out=rf, in_=rf_d[v])
                    st[("nf", v)] = nf
                    st[("rf", v)] = rf

                def phase_c(g):
                    """weighted-sum matmuls for group g (values prefetched)."""
                    em = st.pop(("em", g))
                    xp = xps.tile([128, 2, KC, G], F32, tag="xp")
                    for vj in range(VPJ):
                        nf = st.pop(("nf", g * VPJ + vj))
                        rf = st.pop(("rf", g * VPJ + vj))
                        for i in range(VP):
                            jj = vj * VP + i
                            for c in range(KC):
                                nc.tensor.matmul(
                                    xp[:, 0, c, jj:jj + 1], nf[:, i, c, :],
                                    em[:, 0, jj:jj + 1],
                                    start=True, stop=True)
                            for c in range(KC):
                                nc.tensor.matmul(
                                    xp[:, 1, c, jj:jj + 1], rf[:, i, 0, c, :],
                                    em[:, 1, jj:jj + 1],
                                    start=True, stop=False)
                                nc.tensor.matmul(
                                    xp[:, 1, c, jj:jj + 1], rf[:, i, 1, c, :],
                                    em[:, 2, jj:jj + 1],
                                    start=False, stop=True)
                    g0 = g * G
                    nc.vector.tensor_copy(XTn[:, :, g0:g0 + G], xp[:, 0])
                    nc.vector.tensor_copy(XTr[:, :, g0:g0 + G], xp[:, 1])

                PVLAG = 4  # value stream trails by 4 score blocks
                for g in range(GROUPS):
                    sc = scps.tile([128, 3, G], F32, tag="sc")
                    for j in range(NPJ):
                        blk = g * NPJ + j
                        phase_a(blk)
                        if blk >= PVLAG:
                            prefetch_values(blk - PVLAG)
                        phase_a_compute(blk, sc, j)
                    if g == GROUPS - 1:
                        for v in range(NVBLK - PVLAG, NVBLK):
                            prefetch_values(v)
                        dma(out=Wng_sb, in_=Wng_d)
                        dma(out=bng_sb, in_=bng_d)
                        dma(out=Wrg_sb, in_=Wrg_d)
                        dma(out=brg_sb, in_=brg_d)
                    if g > 0:
                        phase_c(g - 1)
                    phase_b(g, sc)
                phase_c(GROUPS - 1)

            # ---- normalizers: rS columns [BS, 2] (node, rela) ----
            with (
                tc.tile_pool(name="glue", bufs=1) as gp,
                tc.tile_pool(name="glps", bufs=1, space="PSUM") as gps,
                tc.tile_pool(name="trps", bufs=1, space="PSUM") as tps_p,
            ):
                nc.vector.tensor_add(S_sb[:, 1, :], S_sb[:, 1, :], S_sb[:, 2, :])
                rS = gp.tile([1, 2, BS], F32, tag="rs")
                nc.vector.reciprocal(rS, S_sb[:, 0:2, :])
                rS16 = gp.tile([1, 2, BS], F16, tag="rs16")
                nc.vector.tensor_copy(rS16, rS)
                rSc_ps = gps.tile([BS, 2], F32, tag="rscp")
                for k in range(2):
                    nc.tensor.matmul(rSc_ps[:, k:k + 1], rS16[:, k, :],
                                     ones_col[:1, :1], start=True, stop=True)
                rSc = gp.tile([BS, 2], F32, tag="rsc")
                nc.vector.tensor_copy(rSc, rSc_ps)

                # ---- GLU heads (interleaved: head-2's first matmul group
                # does not depend on node_res, so it overlaps head-1's
                # epilogue) ----
                def pmm(dst, lhs, W_sb, c0):
                    for hh in range(2):
                        for c in range(KC):
                            nc.tensor.matmul(dst[:, hh, :], lhs[:, c, :],
                                             W_sb[:, c0 + c, hh, :],
                                             start=(c == 0), stop=(c == KC - 1))

                p1 = gps.tile([BS, 2, 512], F32, tag="p1")
                p2 = gps.tile([BS, 2, 512], F32, tag="p2")
                q1 = gps.tile([BS, 2, 512], F32, tag="q1")
                pmm(p1, XTn, Wng_sb, 0)            # Xn @ Wng_top
                s1 = gp.tile([BS, 2, 512], F32, tag="ngs1")
                nc.vector.scalar_tensor_tensor(
                    out=s1, in0=p1, scalar=rSc[:, 0:1],
                    in1=bng_sb, op0=ALU.mult, op1=ALU.add)
                pmm(p2, XTr, Wng_sb, KC)           # Xr @ Wng_bot
                pmm(q1, XTr, Wrg_sb, 0)            # Xr @ Wrg_top (head 2)
                t1 = gp.tile([BS, 2, 512], F32, tag="rgs1")
                nc.vector.scalar_tensor_tensor(
                    out=t1, in0=q1, scalar=rSc[:, 1:2],
                    in1=brg_sb, op0=ALU.mult, op1=ALU.add)
                nc.vector.scalar_tensor_tensor(
                    out=s1, in0=p2, scalar=rSc[:, 1:2],
                    in1=s1, op0=ALU.mult, op1=ALU.add)
                sig = gp.tile([BS, 512], F32, tag="ngsig")
                nc.scalar.activation(sig, s1[:, 1, :], AF.Sigmoid)
                nres = gp.tile([BS, 512], F32, tag="ngres")
                nc.vector.tensor_mul(nres, s1[:, 0, :], sig)
                dma(out=nres_d, in_=nres)

                # transpose node_res for the second head
                nres16 = gp.tile([BS, D], F16, tag="n16")
                nc.vector.tensor_copy(nres16, nres)
                nresT = gp.tile([128, KC, BS], F16, tag="nrt")
                for c in range(KC):
                    tps = tps_p.tile([128, BS], F16, tag="tps2")
                    nc.tensor.transpose(tps, nres16[:, c * 128:(c + 1) * 128],
                                        ident[:BS, :BS])
                    nc.vector.tensor_copy(nresT[:, c, :], tps)

                q2 = gps.tile([BS, 2, 512], F32, tag="p1")  # reuse p1's bank
                pmm(q2, nresT, Wrg_sb, KC)         # node_res @ Wrg_bot
                nc.vector.tensor_add(t1, t1, q2)
                sig2 = gp.tile([BS, 512], F32, tag="rgsig")
                nc.scalar.activation(sig2, t1[:, 1, :], AF.Sigmoid)
                rres = gp.tile([BS, 512], F32, tag="rgres")
                nc.vector.tensor_mul(rres, t1[:, 0, :], sig2)
                dma(out=rres_d, in_=rres)

    nc.compile()
    return nc


def make_in_maps(inputs):
    """Shard + lay out full inputs into 8 per-core input dicts (host-side)."""
    f16 = np.float16
    f32 = np.float32

    pnf = np.asarray(inputs["p_node_feats"], dtype=f16)
    nf = np.asarray(inputs["node_feats"], dtype=f16)
    prf = np.asarray(inputs["p_rela_feats"], dtype=f16)
    rf = np.asarray(inputs["rela_feats"], dtype=f16)
    h = np.asarray(inputs["h"], dtype=f16)
    am = np.asarray(inputs["att_masks"], dtype=f16)
    rm = np.asarray(inputs["rela_masks"], dtype=f16)

    def shuf_pnf(x):  # [BS,N,D] -> [NBLK,128,PAIR,KC,N]  (d-partition)
        x = x.reshape(NBLK, PAIR, N, KC, 128)
        return np.ascontiguousarray(x.transpose(0, 4, 1, 3, 2))

    def shuf_prf(x):  # [BS,R,D] -> [NBLK,128,PAIR,KC,R]
        x = x.reshape(NBLK, PAIR, R, KC, 128)
        return np.ascontiguousarray(x.transpose(0, 4, 1, 3, 2))

    def shuf_nf(x):  # [BS,N,D] -> [NVBLK,128,VP,KC,128]  (n-partition)
        x = x.reshape(NVBLK, VP, N, KC, 128)
        return np.ascontiguousarray(x.transpose(0, 2, 1, 3, 4))

    def shuf_rf(x):  # [BS,R,D] -> [NVBLK,128,VP,2,KC,128]
        x = x.reshape(NVBLK, VP, 2, 128, KC, 128)
        return np.ascontiguousarray(x.transpose(0, 3, 1, 2, 4, 5))

    def wcols(w):  # [D] -> [128, KC]
        return np.ascontiguousarray(
            np.asarray(w, dtype=f16).reshape(KC, 128).T)

    Wn = np.ascontiguousarray(
        np.asarray(inputs["W_h2node"], dtype=f16).reshape(KC, 128, D)
        .transpose(1, 0, 2))
    Wr = np.ascontiguousarray(
        np.asarray(inputs["W_h2rela"], dtype=f16).reshape(KC, 128, D)
        .transpose(1, 0, 2))
    Wng = np.ascontiguousarray(
        np.asarray(inputs["W_ng"], dtype=f16).reshape(KC2, 128, 2, 512)
        .transpose(1, 0, 2, 3))
    Wrg = np.ascontiguousarray(
        np.asarray(inputs["W_rg"], dtype=f16).reshape(KC2, 128, 2, 512)
        .transpose(1, 0, 2, 3))
    bng = np.ascontiguousarray(np.broadcast_to(
        np.asarray(inputs["b_ng"], dtype=f32), (BS, 2 * D)).reshape(BS, 2, 512))
    brg = np.ascontiguousarray(np.broadcast_to(
        np.asarray(inputs["b_rg"], dtype=f32), (BS, 2 * D)).reshape(BS, 2, 512))

    shared = {
        "w_h2node": Wn, "w_h2rela": Wr,
        "b_h2node": np.ascontiguousarray(
            np.asarray(inputs["b_h2node"], dtype=f32).reshape(KC, 128).T),
        "b_h2rela": np.ascontiguousarray(
            np.asarray(inputs["b_h2rela"], dtype=f32).reshape(KC, 128).T),
        "w1c": wcols(inputs["w_alpha1"]),
        "w2c": wcols(inputs["w_alpha2"]),
        "w_ng": Wng, "w_rg": Wrg, "bias_ng": bng, "bias_rg": brg,
        "ident": np.eye(128, dtype=f16),
        "ones_col": np.ones((128, 1), dtype=f16),
        "ones_row": np.ones((1, 128), dtype=f16),
    }
    in_maps = []
    for cix in range(NCORES):
        s = slice(cix * BS, (cix + 1) * BS)
        mT = np.empty((128, 3, BS), dtype=f16)
        mT[:, 0, :] = am[s].T
        mT[:, 1, :] = rm[s, :128].T
        mT[:, 2, :] = rm[s, 128:].T
        in_maps.append({
            "h": np.ascontiguousarray(h[s]),
            "pnf": shuf_pnf(pnf[s]), "prf": shuf_prf(prf[s]),
            "nf": shuf_nf(nf[s]), "rf": shuf_rf(rf[s]),
            "mT": np.ascontiguousarray(mT), **shared,
        })
    return in_maps


_NC_CACHE = None
LAST_RESULTS = None  # BassKernelResults of the most recent kernel() call


def kernel(**inputs):
    global _NC_CACHE, LAST_RESULTS
    if _NC_CACHE is None:
        _NC_CACHE = build_program()
    nc = _NC_CACHE
    in_maps = make_in_maps(inputs)
    import os
    trace = os.environ.get("BASS_KERNEL_TRACE", "0") == "1"
    res = run_bass_kernel_spmd(nc, in_maps, core_ids=list(range(NCORES)),
                               trace=trace)
    LAST_RESULTS = res
    node_res = np.concatenate([r["node_res"] for r in res.results], axis=0)
    rela_res = np.concatenate([r["rela_res"] for r in res.results], axis=0)
    return node_res, rela_res


# revision 38
# speedup vs baseline: 1.1353x; 1.0064x over previous
"""Trainium2 Bass kernel for nn_Attention_60885456388891 (gnn_message_passing).

Computation (per batch b):
  node_h = h @ W_h2node + b_h2node
  score_n[n] = sum_d tanh(p_node_feats[b,n,d] + node_h[b,d]) * w_alpha1[d]
  node_w = renorm(softmax(score_n) * att_masks)
  node_res_ = sum_n node_w[n] * node_feats[b,n,:]
  (same for relations)
  node_res = glu(cat(node_res_, rela_res_) @ W_ng + b_ng)
  rela_res = glu(cat(rela_res_, node_res) @ W_rg + b_rg)

Strategy: pure data-parallel over batch B=512 across 8 cores (64 batches/core),
all features downcast to fp16 on the host (halves HBM traffic; rel-err ~1e-3
vs the 2e-2 gate).

Per-core pipeline (v4 design):
  - pnf/prf streamed in d-on-partitions layout: broadcast-add of node_h/rela_h
    becomes a per-partition-scalar DVE add (fp16 4x-ish mode), tanh batches
    into large ACT calls.
  - scores via tiny tanh-stationary PE matmuls (FWL fast weight load):
    lhsT = tanh chunk [128d, 128n], rhs = w_alpha chunk col -> score columns
    accumulate in PSUM.  |score| <= sum|w_alpha| ~ 8, so exp() needs no max
    subtraction; masked-exp columns are used UNNORMALIZED as weights, and the
    1/sum(EM) normalizer is folded into the GLU epilogue as a per-row scale.
  - phase C: nf/rf chunk stationary [128n, 128d] x EM column -> X^T columns
    accumulate directly in the k-chunked layout the GLU matmuls consume (no
    row staging, no transposes).
  - GLU: out = (Xn_u @ Wtop) * rSn + (Xr_u @ Wbot) * rSr + bias_bcast,
    a*sigmoid(g).
"""

import numpy as np

import concourse.bass as bass
import concourse.bacc as bacc
import concourse.mybir as mybir
import concourse.tile as tile
from concourse.bass_utils import run_bass_kernel_spmd

# Problem dims (hardcoded per contract)
B, N, R, D = 512, 128, 256, 512
NCORES = 8
BS = B // NCORES          # 64 batches per core
PAIR = 4                  # batches per stream DMA block
NBLK = BS // PAIR         # 16 blocks
G = 16                    # batches per softmax/psum group
GROUPS = BS // G          # 4 groups
KC = D // 128             # 4 k-chunks of 128
KC2 = 2 * D // 128        # 8 k-chunks for the 1024-wide GLU matmuls
VP = PAIR                 # batches per value-stream DMA block
NVBLK = BS // VP          # value blocks
VPJ = G // VP             # value blocks per group

F32 = mybir.dt.float32
F16 = mybir.dt.float16
AF = mybir.ActivationFunctionType
ALU = mybir.AluOpType
AX = mybir.AxisListType


def build_program():
    nc = bacc.Bacc("TRN2", target_bir_lowering=False, debug=False)

    def din(name, shape, dt=F16):
        return nc.dram_tensor(name, shape, dt, kind="ExternalInput").ap()

    h_d = din("h", [BS, D])
    pnf_d = din("pnf", [NBLK, 128, PAIR, KC, N])        # d-partition args
    prf_d = din("prf", [NBLK, 128, PAIR, KC, R])
    nf_d = din("nf", [NVBLK, 128, VP, KC, 128])         # n-partition values
    rf_d = din("rf", [NVBLK, 128, VP, 2, KC, 128])
    mT_d = din("mT", [128, 3, BS])                      # masks, transposed
    Wn_d = din("w_h2node", [128, KC, D])
    bn_d = din("b_h2node", [128, KC], F32)
    Wr_d = din("w_h2rela", [128, KC, D])
    br_d = din("b_h2rela", [128, KC], F32)
    w1_d = din("w1c", [128, KC])                        # w_alpha1 as columns
    w2_d = din("w2c", [128, KC])
    Wng_d = din("w_ng", [128, KC2, 2, 512])
    Wrg_d = din("w_rg", [128, KC2, 2, 512])
    bng_d = din("bias_ng", [BS, 2, 512], F32)           # host-broadcast bias
    brg_d = din("bias_rg", [BS, 2, 512], F32)
    id_d = din("ident", [128, 128])                     # f16 identity
    ones_d = din("ones_col", [128, 1])                  # f16 ones column
    onesr_d = din("ones_row", [1, 128])                 # f16 ones row

    nres_d = nc.dram_tensor("node_res", [BS, D], F32, kind="ExternalOutput").ap()
    rres_d = nc.dram_tensor("rela_res", [BS, D], F32, kind="ExternalOutput").ap()

    dma = nc.sync.dma_start
    dma_s = nc.gpsimd.dma_start

    with tile.TileContext(nc) as tc:
        with (
            tc.tile_pool(name="const", bufs=1) as cp,
        ):
            # ---- persistent constants ----
            ident = cp.tile([128, 128], F16)
            dma(out=ident, in_=id_d)
            ones_col = cp.tile([128, 1], F16)
            dma(out=ones_col, in_=ones_d)
            ones_row = cp.tile([1, 128], F16)
            dma(out=ones_row, in_=onesr_d)
            w1c = cp.tile([128, KC], F16)
            dma(out=w1c, in_=w1_d)
            w2c = cp.tile([128, KC], F16)
            dma(out=w2c, in_=w2_d)
            mT = cp.tile([128, 3, BS], F16)
            dma(out=mT, in_=mT_d)

            # persistent outputs of phase C / B
            XTn = cp.tile([128, KC, BS], F16, tag="xtn")   # unnormalized Xn^T
            XTr = cp.tile([128, KC, BS], F16, tag="xtr")
            S_sb = cp.tile([1, 3, BS], F32, tag="ssb")     # EM column sums
            nhT = cp.tile([128, KC, BS], F32, tag="nht")   # bias columns
            rhT = cp.tile([128, KC, BS], F32, tag="rht")
            # GLU weights/biases (DMA'd during the last group's streaming)
            Wng_sb = cp.tile([128, KC2, 2, 512], F16, tag="wng")
            Wrg_sb = cp.tile([128, KC2, 2, 512], F16, tag="wrg")
            bng_sb = cp.tile([BS, 2, 512], F32, tag="bng")
            brg_sb = cp.tile([BS, 2, 512], F32, tag="brg")

            # ---- prologue: nhT/rhT bias columns = (h @ W + b)^T, computed
            # directly in transposed chunk layout (no row round-trip) ----
            with (
                tc.tile_pool(name="prol", bufs=1) as pp,
                tc.tile_pool(name="prps", bufs=2, space="PSUM") as pps,
            ):
                h_sb = pp.tile([BS, D], F16, tag="h")
                dma(out=h_sb, in_=h_d)
                Wn_sb = pp.tile([128, KC, D], F16, tag="wn")
                dma(out=Wn_sb, in_=Wn_d)
                Wr_sb = pp.tile([128, KC, D], F16, tag="wr")
                dma(out=Wr_sb, in_=Wr_d)
                bn_sb = pp.tile([128, KC], F32, tag="bn")
                dma(out=bn_sb, in_=bn_d)
                br_sb = pp.tile([128, KC], F32, tag="br")
                dma(out=br_sb, in_=br_d)

                hT = pp.tile([128, KC, BS], F16, tag="ht")
                for c in range(KC):
                    tps = pps.tile([128, BS], F16, tag="tps")
                    nc.tensor.transpose(tps, h_sb[:, c * 128:(c + 1) * 128],
                                        ident[:BS, :BS])
                    nc.vector.tensor_copy(hT[:, c, :], tps)
                for W_sb, b_sb, dstT in ((Wn_sb, bn_sb, nhT),
                                         (Wr_sb, br_sb, rhT)):
                    for c in range(KC):
                        ps = pps.tile([128, BS], F32, tag="nhcps")
                        for k in range(KC):
                            nc.tensor.matmul(
                                ps, W_sb[:, k, c * 128:(c + 1) * 128],
                                hT[:, k, :],
                                start=(k == 0), stop=(k == KC - 1))
                        nc.vector.tensor_scalar_add(dstT[:, c, :], ps,
                                                    b_sb[:, c:c + 1])

            # ---- main loop: per-PAIR software pipeline over 16 slots ----
            # Slot k: [pnf/prf DMA (k)] [exp/mask/S (k-1)] [phase-C mm (k-2)]
            #         [adds+tanh (k)] [score mm (k)] [nf/rf prefetch DMA (k)]
            with (
                tc.tile_pool(name="pnfp", bufs=7) as pnfp,
                tc.tile_pool(name="prfp", bufs=7) as prfp,
                tc.tile_pool(name="nfp", bufs=6) as nfp,
                tc.tile_pool(name="rfp", bufs=6) as rfp,
                tc.tile_pool(name="emp", bufs=2) as emp,
                tc.tile_pool(name="scps", bufs=2, space="PSUM") as scps,
                tc.tile_pool(name="xps", bufs=2, space="PSUM") as xps,
                tc.tile_pool(name="sps", bufs=2, space="PSUM") as sps,
            ):
                NPJ = G // PAIR       # pair slots per group
                NSLOT = NBLK          # total slots
                LAG = 2               # phase-C runs LAG slots behind phase-A
                st = {}               # per-slot live tiles

                def phase_a(k):
                    pnf = pnfp.tile([128, PAIR, KC, N], F16, tag="pnf")
                    dma(out=pnf, in_=pnf_d[k])
                    prf = prfp.tile([128, PAIR, KC, R], F16, tag="prf")
                    dma(out=prf, in_=prf_d[k])
                    st[("pnf", k)] = pnf
                    st[("prf", k)] = prf

                def phase_a_compute(k, sc, j):
                    pnf = st.pop(("pnf", k))
                    prf = st.pop(("prf", k))
                    for i in range(PAIR):
                        b = k * PAIR + i
                        for c in range(KC):
                            nc.vector.tensor_scalar_add(
                                pnf[:, i, c, :], pnf[:, i, c, :],
                                nhT[:, c, b:b + 1])
                            nc.vector.tensor_scalar_add(
                                prf[:, i, c, :], prf[:, i, c, :],
                                rhT[:, c, b:b + 1])
                    nc.scalar.activation(pnf, pnf, AF.Tanh)
                    nc.scalar.activation(prf, prf, AF.Tanh)
                    for i in range(PAIR):
                        jj = j * PAIR + i
                        for c in range(KC):
                            nc.tensor.matmul(
                                sc[:, 0, jj:jj + 1], pnf[:, i, c, :],
                                w1c[:, c:c + 1],
                                start=(c == 0), stop=(c == KC - 1))
                        for c in range(KC):
                            nc.tensor.matmul(
                                sc[:, 1, jj:jj + 1], prf[:, i, c, :128],
                                w2c[:, c:c + 1],
                                start=(c == 0), stop=(c == KC - 1))
                        for c in range(KC):
                            nc.tensor.matmul(
                                sc[:, 2, jj:jj + 1], prf[:, i, c, 128:],
                                w2c[:, c:c + 1],
                                start=(c == 0), stop=(c == KC - 1))

                def phase_b(g, sc):
                    """masked exp + column sums for group g."""
                    g0 = g * G
                    em = emp.tile([128, 3, G], F16, tag="em")
                    st[("em", g)] = em
                    nc.scalar.activation(em, sc, AF.Exp)
                    nc.vector.tensor_mul(em, em, mT[:, :, g0:g0 + G])
                    s_ps = sps.tile([1, 3, G], F32, tag="s")
                    nc.tensor.matmul(s_ps, ones_col, em, start=True, stop=True)
                    nc.vector.tensor_copy(S_sb[:, :, g0:g0 + G], s_ps)

                def prefetch_values(v):
                    nf = nfp.tile([128, VP, KC, 128], F16, tag="nf")
                    dma(out=nf, in_=nf_d[v])
                    rf = rfp.tile([128, VP, 2, KC, 128], F16, tag="rf")
                    dma(out=rf, in_=rf_d[v])
                    st[("nf", v)] = nf
                    st[("rf", v)] = rf

                def phase_c(g):
                    """weighted-sum matmuls for group g (values prefetched)."""
                    em = st.pop(("em", g))
                    xp = xps.tile([128, 2, KC, G], F32, tag="xp")
                    for vj in range(VPJ):
                        nf = st.pop(("nf", g * VPJ + vj))
                        rf = st.pop(("rf", g * VPJ + vj))
                        for i in range(VP):
                            jj = vj * VP + i
                            for c in range(KC):
                                nc.tensor.matmul(
                                    xp[:, 0, c, jj:jj + 1], nf[:, i, c, :],
                                    em[:, 0, jj:jj + 1],
                                    start=True, stop=True)
                            for c in range(KC):
                                nc.tensor.matmul(
                                    xp[:, 1, c, jj:jj + 1], rf[:, i, 0, c, :],
                                    em[:, 1, jj:jj + 1],
                                    start=True, stop=False)
                                nc.tensor.matmul(
                                    xp[:, 1, c, jj:jj + 1], rf[:, i, 1, c, :],
                                    em[:, 2, jj:jj + 1],
                                    start=False, stop=True)
                    g0 = g * G
                    nc.vector.tensor_copy(XTn[:, :, g0:g0 + G], xp[:, 0])
                    nc.vector.tensor_copy(XTr[:, :, g0:g0 + G], xp[:, 1])

                PVLAG = 4  # value stream trails by 4 score blocks
                for g in range(GROUPS):
                    sc = scps.tile([128, 3, G], F32, tag="sc")
                    for j in range(NPJ):
                        blk = g * NPJ + j
                        phase_a(blk)
                        if blk >= PVLAG:
                            prefetch_values(blk - PVLAG)
                        phase_a_compute(blk, sc, j)
                    if g == GROUPS - 1:
                        for v in range(NVBLK - PVLAG, NVBLK):
                            prefetch_values(v)
                        dma(out=Wng_sb, in_=Wng_d)
                        dma(out=bng_sb, in_=bng_d)
                        dma(out=Wrg_sb, in_=Wrg_d)
                        dma(out=brg_sb, in_=brg_d)
                    if g > 0:
                        phase_c(g - 1)
                    phase_b(g, sc)
                phase_c(GROUPS - 1)

            # ---- normalizers: rS columns [BS, 2] (node, rela) ----
            with (
                tc.tile_pool(name="glue", bufs=1) as gp,
                tc.tile_pool(name="glps", bufs=1, space="PSUM") as gps,
                tc.tile_pool(name="trps", bufs=1, space="PSUM") as tps_p,
            ):
                nc.vector.tensor_add(S_sb[:, 1, :], S_sb[:, 1, :], S_sb[:, 2, :])
                rS = gp.tile([1, 2, BS], F32, tag="rs")
                nc.vector.reciprocal(rS, S_sb[:, 0:2, :])
                rS16 = gp.tile([1, 2, BS], F16, tag="rs16")
                nc.vector.tensor_copy(rS16, rS)
                rSc_ps = gps.tile([BS, 2], F32, tag="rscp")
                for k in range(2):
                    nc.tensor.matmul(rSc_ps[:, k:k + 1], rS16[:, k, :],
                                     ones_col[:1, :1], start=True, stop=True)
                rSc = gp.tile([BS, 2], F32, tag="rsc")
                nc.vector.tensor_copy(rSc, rSc_ps)

                # ---- GLU heads (interleaved: head-2's first matmul group
                # does not depend on node_res, so it overlaps head-1's
                # epilogue) ----
                def pmm(dst, lhs, W_sb, c0):
                    for hh in range(2):
                        for c in range(KC):
                            nc.tensor.matmul(dst[:, hh, :], lhs[:, c, :],
                                             W_sb[:, c0 + c, hh, :],
                                             start=(c == 0), stop=(c == KC - 1))

                p1 = gps.tile([BS, 2, 512], F32, tag="p1")
                p2 = gps.tile([BS, 2, 512], F32, tag="p2")
                q1 = gps.tile([BS, 2, 512], F32, tag="q1")
                pmm(p1, XTn, Wng_sb, 0)            # Xn @ Wng_top
                s1 = gp.tile([BS, 2, 512], F32, tag="ngs1")
                nc.vector.scalar_tensor_tensor(
                    out=s1, in0=p1, scalar=rSc[:, 0:1],
                    in1=bng_sb, op0=ALU.mult, op1=ALU.add)
                pmm(p2, XTr, Wng_sb, KC)           # Xr @ Wng_bot
                pmm(q1, XTr, Wrg_sb, 0)            # Xr @ Wrg_top (head 2)
                t1 = gp.tile([BS, 2, 512], F32, tag="rgs1")
                nc.vector.scalar_tensor_tensor(
                    out=t1, in0=q1, scalar=rSc[:, 1:2],
                    in1=brg_sb, op0=ALU.mult, op1=ALU.add)
                nc.vector.scalar_tensor_tensor(
                    out=s1, in0=p2, scalar=rSc[:, 1:2],
                    in1=s1, op0=ALU.mult, op1=ALU.add)
                sig = gp.tile([BS, 512], F32, tag="ngsig")
                nc.scalar.activation(sig, s1[:, 1, :], AF.Sigmoid)
                nres = gp.tile([BS, 512], F32, tag="ngres")
                nc.vector.tensor_mul(nres, s1[:, 0, :], sig)
                dma(out=nres_d, in_=nres)

                # transpose node_res for the second head
                nres16 = gp.tile([BS, D], F16, tag="n16")
                nc.vector.tensor_copy(nres16, nres)
                nresT = gp.tile([128, KC, BS], F16, tag="nrt")
                for c in range(KC):
                    tps = tps_p.tile([128, BS], F16, tag="tps2")
                    nc.tensor.transpose(tps, nres16[:, c * 128:(c + 1) * 128],
                                        ident[:BS, :BS])
                    nc.vector.tensor_copy(nresT[:, c, :], tps)

                q2 = gps.tile([BS, 2, 512], F32, tag="p1")  # reuse p1's bank
                pmm(q2, nresT, Wrg_sb, KC)         # node_res @ Wrg_bot
                nc.vector.tensor_add(t1, t1, q2)
                sig2 = gp.tile([BS, 512], F32, tag="rgsig")
                nc.scalar.activation(sig2, t1[:, 1, :], AF.Sigmoid)
                rres = gp.tile([BS, 512], F32, tag="rgres")
                nc.vector.tensor_mul(rres, t1[:, 0, :], sig2)
                dma(out=rres_d, in_=rres)

    nc.compile()
    return nc


def make_in_maps(inputs):
    """Shard + lay out full inputs into 8 per-core input dicts (host-side)."""
    f16 = np.float16
    f32 = np.float32

    pnf = np.asarray(inputs["p_node_feats"], dtype=f16)
    nf = np.asarray(inputs["node_feats"], dtype=f16)
    prf = np.asarray(inputs["p_rela_feats"], dtype=f16)
    rf = np.asarray(inputs["rela_feats"], dtype=f16)
    h = np.asarray(inputs["h"], dtype=f16)
    am = np.asarray(inputs["att_masks"], dtype=f16)
    rm = np.asarray(inputs["rela_masks"], dtype=f16)

    def shuf_pnf(x):  # [BS,N,D] -> [NBLK,128,PAIR,KC,N]  (d-partition)
        x = x.reshape(NBLK, PAIR, N, KC, 128)
        return np.ascontiguousarray(x.transpose(0, 4, 1, 3, 2))

    def shuf_prf(x):  # [BS,R,D] -> [NBLK,128,PAIR,KC,R]
        x = x.reshape(NBLK, PAIR, R, KC, 128)
        return np.ascontiguousarray(x.transpose(0, 4, 1, 3, 2))

    def shuf_nf(x):  # [BS,N,D] -> [NVBLK,128,VP,KC,128]  (n-partition)
        x = x.reshape(NVBLK, VP, N, KC, 128)
        return np.ascontiguousarray(x.transpose(0, 2, 1, 3, 4))

    def shuf_rf(x):  # [BS,R,D] -> [NVBLK,128,VP,2,KC,128]
        x = x.reshape(NVBLK, VP, 2, 128, KC, 128)
        return np.ascontiguousarray(x.transpose(0, 3, 1, 2, 4, 5))

    def wcols(w):  # [D] -> [128, KC]
        return np.ascontiguousarray(
            np.asarray(w, dtype=f16).reshape(KC, 128).T)

    Wn = np.ascontiguousarray(
        np.asarray(inputs["W_h2node"], dtype=f16).reshape(KC, 128, D)
        .transpose(1, 0, 2))
    Wr = np.ascontiguousarray(
        np.asarray(inputs["W_h2rela"], dtype=f16).reshape(KC, 128, D)
        .transpose(1, 0, 2))
    Wng = np.ascontiguousarray(
        np.asarray(inputs["W_ng"], dtype=f16).reshape(KC2, 128, 2, 512)
        .transpose(1, 0, 2, 3))
    Wrg = np.ascontiguousarray(
        np.asarray(inputs["W_rg"], dtype=f16).reshape(KC2, 128, 2, 512)
        .transpose(1, 0, 2, 3))
    bng = np.ascontiguousarray(np.broadcast_to(
        np.asarray(inputs["b_ng"], dtype=f32), (BS, 2 * D)).reshape(BS, 2, 512))
    brg = np.ascontiguousarray(np.broadcast_to(
        np.asarray(inputs["b_rg"], dtype=f32), (BS, 2 * D)).reshape(BS, 2, 512))

    shared = {
        "w_h2node": Wn, "w_h2rela": Wr,
        "b_h2node": np.ascontiguousarray(
            np.asarray(inputs["b_h2node"], dtype=f32).reshape(KC, 128).T),
        "b_h2rela": np.ascontiguousarray(
            np.asarray(inputs["b_h2rela"], dtype=f32).reshape(KC, 128).T),
        "w1c": wcols(inputs["w_alpha1"]),
        "w2c": wcols(inputs["w_alpha2"]),
        "w_ng": Wng, "w_rg": Wrg, "bias_ng": bng, "bias_rg": brg,
        "ident": np.eye(128, dtype=f16),
        "ones_col": np.ones((128, 1), dtype=f16),
        "ones_row": np.ones((1, 128), dtype=f16),
    }
    in_maps = []
    for cix in range(NCORES):
        s = slice(cix * BS, (cix + 1) * BS)
        mT = np.empty((128, 3, BS), dtype=f16)
        mT[:, 0, :] = am[s].T
        mT[:, 1, :] = rm[s, :128].T
        mT[:, 2, :] = rm[s, 128:].T
        in_maps.append({
            "h": np.ascontiguousarray(h[s]),
            "pnf": shuf_pnf(pnf[s]), "prf": shuf_prf(prf[s]),
            "nf": shuf_nf(nf[s]), "rf": shuf_rf(rf[s]),
            "mT": np.ascontiguousarray(mT), **shared,
        })
    return in_maps


_NC_CACHE = None
LAST_RESULTS = None  # BassKernelResults of the most recent kernel() call


def kernel(**inputs):
    global _NC_CACHE, LAST_RESULTS
    if _NC_CACHE is None:
        _NC_CACHE = build_program()
    nc = _NC_CACHE
    in_maps = make_in_maps(inputs)
    import os
    trace = os.environ.get("BASS_KERNEL_TRACE", "0") == "1"
    res = run_bass_kernel_spmd(nc, in_maps, core_ids=list(range(NCORES)),
                               trace=trace)
    LAST_RESULTS = res
    node_res = np.concatenate([r["node_res"] for r in res.results], axis=0)
    rela_res = np.concatenate([r["rela_res"] for r in res.results], axis=0)
    return node_res, rela_res
